# revision 1
# baseline (speedup 1.0000x reference)
"""Trainium2 Bass kernel for NeuralGraphHidden (GNN message passing).

Math (per molecule b, atom a):
    deg[b,a]    = #valid edges (edges[b,a,:] != -1)
    summed_atom = atoms[b,a] + sum_s atoms[b, edges[b,a,s]]          (64)
    bond_sum    = sum_s bonds[b,a,s]                                  (8)
    x           = concat(summed_atom, bond_sum)                      (72)
    out[b,a]    = relu(x @ Ws[deg] + bs[deg])  if deg <= 5 else 0   (128)

Design notes (driven by measured TRN2 behaviour on this system):
  * Every device-side random-row gather mechanism measured 20-500 ns/row
    (Ant dma_gather HBM ~49 ns/idx, SBUF-source ~500 ns/idx, generic
    indirect ~300 ns/row at its supported [128,1]-offset granularity, and
    wide offset APs silently corrupt data on HW).  At ~50k gathered rows
    per core that is milliseconds - 30x over the memory roofline.  The
    host therefore performs all *layout* work (degree-sort permutation,
    neighbour row expansion via np.take, bf16 packing), which is pure
    indexed data movement, and the device does all arithmetic: neighbour
    summation, transposes, per-degree dense layers, relu.
  * Pure data parallel: 128 molecules per core (8 cores), one SPMD
    program; per-degree groups padded to a fixed 2560 slots so all cores
    share it.
  * Device pipeline, all contiguous DMA:
      1. load degree-sorted token rows [atoms|bonds] (bf16, HWDGE)
      2. load expanded neighbour atom rows (bf16, HWDGE); slot-s list is
         a prefix of the degree-DESC sorted order
      3. DVE adds accumulate neighbour sums into the self rows
      4. per 128-token tile: PE transpose -> [feature, token]; one matmul
         against the tile's degree weights + a K=1 bias matmul
      5. relu on ScalarE -> bf16 sorted output rows (HWDGE store)
  * Host unpermutes the sorted output (deg-6 rows are zero).
"""

import sys

sys.path.insert(0, "/opt/trn_rl_repo")

import numpy as np
import ml_dtypes

from contextlib import ExitStack

import concourse.bacc as bacc
import concourse.tile as tile
from concourse import mybir
from concourse.bass_utils import run_bass_kernel_spmd
from concourse.masks import make_identity

# Problem shapes (hardcoded per the harness contract).
B, A, D = 1024, 128, 6
F_ATOM, F_BOND, CONV = 64, 8, 128
FAN_IN = F_ATOM + F_BOND  # 72
NCORES = 8
BS = B // NCORES          # molecules per core = 128
T = BS * A                # tokens per core = 16384
ROW = F_ATOM + D * F_BOND               # 112 features per packed row
GROUP_PAD = 2560                        # per-degree group size (static)
NSORT = D * GROUP_PAD                   # 15360 sorted slots
KT = NSORT // 128                       # 120 token tiles
KG = GROUP_PAD // 128                   # 20 tiles per degree group
# neighbour slot-s list covers sorted slots [0, PREFIX[s]) (degree-DESC)
PREFIX = [(D - 1 - s) * GROUP_PAD for s in range(D - 1)]
NCOL = [p // 128 for p in PREFIX]       # offset columns per slot: 100,80,...
SOFF = [0]
for n in NCOL:
    SOFF.append(SOFF[-1] + n)
NTOT = SOFF[-1]                         # 300 neighbour columns overall

_f32 = mybir.dt.float32
_bf16 = mybir.dt.bfloat16

_cached = {}


def build_program(repeat=1, stages="laxmr"):
    """Build the (static) per-core Bass/Tile program.

    stages: subset of l(oads) a(dds) x(transpose) m(atmul) r(elu+store)."""
    nc = bacc.Bacc("TRN2", target_bir_lowering=False, debug=False)

    xrows = nc.dram_tensor("xrows", [128, KT * ROW], _bf16,
                           kind="ExternalInput")
    nrows = nc.dram_tensor("nrows", [128, NTOT * F_ATOM], _bf16,
                           kind="ExternalInput")
    wfull = nc.dram_tensor("wfull", [D, ROW, CONV], _bf16, kind="ExternalInput")
    bsrow = nc.dram_tensor("bsrow", [D, 1, CONV], _bf16, kind="ExternalInput")
    osort = nc.dram_tensor("osort", [128, KT * CONV], _bf16,
                           kind="ExternalOutput")

    with tile.TileContext(nc) as tc, ExitStack() as ctx:
        const_pool = ctx.enter_context(tc.tile_pool(name="const", bufs=1))
        work_pool = ctx.enter_context(tc.tile_pool(name="work", bufs=1))
        xt_pool = ctx.enter_context(tc.tile_pool(name="xt", bufs=4))
        ps_pool = ctx.enter_context(tc.tile_pool(name="ps", bufs=3, space="PSUM"))
        pt_pool = ctx.enter_context(tc.tile_pool(name="pt", bufs=3, space="PSUM"))

        wfull_t, bs_t = [], []
        for d in range(D):
            wf = const_pool.tile([ROW, CONV], _bf16, tag=f"w{d}")
            nc.sync.dma_start(out=wf[:], in_=wfull[d])
            wfull_t.append(wf)
            bt = const_pool.tile([1, CONV], _bf16, tag=f"b{d}")
            nc.sync.dma_start(out=bt[:], in_=bsrow[d])
            bs_t.append(bt)
        ones = const_pool.tile([1, 128], _bf16, tag="ones")
        nc.vector.memset(ones[:], 1.0)
        ident = const_pool.tile([128, 128], _bf16, tag="ident")
        make_identity(nc, ident[:])

        for rep in range(repeat):
            # 1+2. contiguous loads (token k*128+p lives at [p, k])
            selfsb = work_pool.tile([128, KT, ROW], _bf16, tag="selfsb")
            neigh = work_pool.tile([128, NTOT, F_ATOM], _bf16, tag="neigh")
            if "l" in stages:
                nc.sync.dma_start(
                    out=selfsb[:],
                    in_=xrows[:].rearrange("p (k e) -> p k e", e=ROW))
                nc.sync.dma_start(
                    out=neigh[:],
                    in_=nrows[:].rearrange("p (k e) -> p k e", e=F_ATOM))
            elif rep == 0:
                nc.vector.memset(selfsb[:], 0.25)
                nc.vector.memset(neigh[:], 0.25)

            # 3-5. per-tile: neighbour adds, transpose, matmul, relu
            outsb = work_pool.tile([128, KT, CONV], _bf16, tag="outsb")
            if "r" not in stages and rep == 0:
                nc.vector.memset(outsb[:], 0.5)
            for k in range(KT):
                d = D - 1 - (k // KG)          # tile degree (DESC order)
                if "a" in stages:
                    for s in range(d):
                        nc.vector.tensor_add(
                            selfsb[:, k, 0:F_ATOM],
                            selfsb[:, k, 0:F_ATOM],
                            neigh[:, SOFF[s] + k, :],
                        )
                if "x" in stages:
                    pt = pt_pool.tile([ROW, 128], _bf16, tag="pt")
                    nc.tensor.transpose(out=pt[:], in_=selfsb[:, k, :],
                                        identity=ident[:])
                    xt = xt_pool.tile([ROW, 128], _bf16, tag="xtt")
                    nc.vector.tensor_copy(xt[:], pt[:])
                else:
                    xt = None
                if "m" in stages and xt is not None:
                    ps = ps_pool.tile([128, CONV], _f32, tag="ps")
                    nc.tensor.matmul(out=ps[:], lhsT=xt[:],
                                     rhs=wfull_t[d][:],
                                     start=True, stop=False)
                    nc.tensor.matmul(out=ps[:], lhsT=ones[:], rhs=bs_t[d][:],
                                     start=False, stop=True)
                    if "r" in stages:
                        nc.scalar.activation(
                            outsb[:, k, :], ps[:],
                            mybir.ActivationFunctionType.Relu)

            if "r" in stages:
                nc.sync.dma_start(
                    out=osort[:].rearrange("p (k e) -> p k e", e=CONV),
                    in_=outsb[:])

    nc.compile()
    return nc


def _get_program():
    if "nc" not in _cached:
        _cached["nc"] = build_program()
    return _cached["nc"]


def prep_core_inputs(atoms_s, bonds_s, edges_s, wfull_np, bsrow_np):
    """Host-side layout/index prep for one core's shard (numpy only)."""
    deg = (edges_s != -1).sum(axis=-1).reshape(-1)            # [T] natural
    slot_tok = np.full(NSORT, -1, np.int64)   # sorted slot -> natural token
    for d in range(D):
        toks = np.nonzero(deg == d)[0]
        n = len(toks)
        assert n <= GROUP_PAD, f"degree-{d} group has {n} > {GROUP_PAD}"
        base = (D - 1 - d) * GROUP_PAD
        slot_tok[base:base + n] = toks

    flat = np.concatenate(
        [atoms_s.reshape(T, F_ATOM), bonds_s.reshape(T, D * F_BOND)], axis=1
    ).astype(ml_dtypes.bfloat16)                              # [T, 112]
    safe = np.maximum(slot_tok, 0)
    xrows = np.where((slot_tok >= 0)[:, None], flat[safe],
                     ml_dtypes.bfloat16(0))                   # [NSORT, 112]
    # slot j -> [partition j%128, tile j//128]
    xrows = xrows.reshape(KT, 128, ROW).transpose(1, 0, 2).reshape(128, -1)

    eflat = edges_s.reshape(T, D)
    bcol = (np.arange(T) // A) * A                            # molecule base
    atoms_flat = flat[:, :F_ATOM]
    ncols = []
    for s in range(D - 1):
        slots = slot_tok[:PREFIX[s]]
        svalid = slots >= 0
        e = np.where(svalid, eflat[np.maximum(slots, 0), s], -1)
        nat = np.maximum(bcol[np.maximum(slots, 0)] + e, 0)
        rows = np.where((e >= 0)[:, None], atoms_flat[nat],
                        ml_dtypes.bfloat16(0))                # [PREFIX[s], 64]
        ncols.append(rows.reshape(NCOL[s], 128, F_ATOM))
    nrows = np.concatenate(ncols, axis=0)                     # [NTOT,128,64]
    nrows = nrows.transpose(1, 0, 2).reshape(128, -1)

    return {
        "xrows": np.ascontiguousarray(xrows),
        "nrows": np.ascontiguousarray(nrows),
        "wfull": wfull_np,
        "bsrow": bsrow_np,
    }, slot_tok


def kernel(atoms, bonds, edges, Ws, bs, trace=False):
    atoms = np.asarray(atoms)
    bonds = np.asarray(bonds)
    edges = np.asarray(edges)
    Ws = np.asarray(Ws)
    bs = np.asarray(bs)

    # Wfull rows = [Wa (64) | tile(Wb, 6) (48)]; bias via K=1 ones matmul
    wfull_np = np.zeros((D, ROW, CONV), np.float32)
    wfull_np[:, :F_ATOM] = Ws[:, :F_ATOM]
    wfull_np[:, F_ATOM:] = np.tile(Ws[:, F_ATOM:], (1, D, 1))
    wfull_np = wfull_np.astype(ml_dtypes.bfloat16)
    bsrow_np = bs.reshape(D, 1, CONV).astype(ml_dtypes.bfloat16)

    in_maps, slot_toks = [], []
    for c in range(NCORES):
        sl = slice(c * BS, (c + 1) * BS)
        m, st = prep_core_inputs(atoms[sl], bonds[sl], edges[sl],
                                 wfull_np, bsrow_np)
        in_maps.append(m)
        slot_toks.append(st)

    nc = _get_program()
    res = run_bass_kernel_spmd(nc, in_maps, core_ids=list(range(NCORES)),
                               trace=trace)
    kernel.last_results = res

    out = np.zeros((B, A, CONV), np.float32)
    for c in range(NCORES):
        osort = res.results[c]["osort"].view(ml_dtypes.bfloat16)
        osort = osort.reshape(128, KT, CONV).transpose(1, 0, 2).reshape(
            NSORT, CONV)                                      # slot-major
        st = slot_toks[c]
        real = st >= 0
        shard = out[c * BS:(c + 1) * BS].reshape(T, CONV)
        shard[st[real]] = osort[real].astype(np.float32)
    return out



# revision 10
# speedup vs baseline: 2.6411x; 2.6411x over previous
"""Trainium2 Bass kernel for NeuralGraphHidden (GNN message passing).

Math (per molecule b, atom a):
    deg[b,a]    = #valid edges (edges[b,a,:] != -1)
    summed_atom = atoms[b,a] + sum_s atoms[b, edges[b,a,s]]          (64)
    bond_sum    = sum_s bonds[b,a,s]                                  (8)
    x           = concat(summed_atom, bond_sum)                      (72)
    out[b,a]    = relu(x @ Ws[deg] + bs[deg])  if deg <= 5 else 0   (128)

Design (v3 — feature-major layout, everything folds into PE accumulation):
  * Host does all *layout* work (degree-sort permutation, neighbour row
    expansion via np.take, transposition to [feature, token] order, bf16
    packing) — pure indexed data movement.  Device does all arithmetic.
  * Feature-major: tokens are matmul rhs columns, so NO on-device
    transposes.  out^T[conv, tok] = W_d^T @ x^T with W_d the stationary
    operand, N=512 moving tiles.
  * Bond features are shipped raw (48 = 6 slots x 8); the bond-slot sum
    folds into the matmul via W rows 64:112 = tile(Wb, 6).
  * Neighbour atom rows (degree-DESC sorted, slot-s list is a prefix)
    are packed two 64-row slabs per 128 partitions.  The slab summation
    folds into the SAME PSUM accumulation via duplicated atom weights:
      [top; bottom] @ [Wa_d; Wa_d] == (top + bottom) @ Wa_d
    (DVE can't add across partition halves — lanes are partition-locked
    — but the PE contraction dimension can.)  Odd slab counts zero-pad
    the last pair's bottom half (~1MB extra traffic) so every matmul is
    a uniform K=128 base-partition-0 full-bank accumulate — half-K
    base-64 row-group matmuls crash on HW in this stack.
  * Bias + relu + f32->bf16 fold into one ScalarE activation per chunk
    (bias is a per-partition [128,1] AP in the conv-major layout).
  * Per-degree groups padded to 2560 slots so all 8 cores share one
    SPMD program; host unpermutes the sorted output (deg-6 rows zero).
"""

import sys

sys.path.insert(0, "/opt/trn_rl_repo")

import numpy as np
import ml_dtypes

from contextlib import ExitStack

import concourse.bacc as bacc
import concourse.tile as tile
from concourse import mybir
from concourse.bass_utils import run_bass_kernel_spmd

# Problem shapes (hardcoded per the harness contract).
B, A, D = 1024, 128, 6
F_ATOM, F_BOND, CONV = 64, 8, 128
FAN = F_ATOM + D * F_BOND               # 112 features per packed column
NCORES = 8
BS = B // NCORES          # molecules per core = 128
T = BS * A                # tokens per core = 16384
GROUP_PAD = 2560                        # per-degree group size (static)
HALF = GROUP_PAD // 2                   # 1280
NSORT = D * GROUP_PAD                   # 15360 sorted slots
CHUNK = 512                             # matmul moving-tile width
NCHUNK = GROUP_PAD // CHUNK             # 5 chunks per degree group
# neighbour block for group g (degree d=5-g): ceil(d/2) pair sections
NPAIR = [(max(D - 1 - g, 0) + 1) // 2 for g in range(D)]
NWIDTH = [p * GROUP_PAD for p in NPAIR]
NOFF = [sum(NWIDTH[:g]) for g in range(D)]
NTOTC = sum(NWIDTH)                     # 23040 packed neighbour columns

_f32 = mybir.dt.float32
_bf16 = mybir.dt.bfloat16

_cached = {}

import os
_VARIANT = os.environ.get("K_VARIANT", "spt")  # s(elf) p(airs) t(ails)


def build_program():
    """Build the (static) per-core Bass/Tile program."""
    nc = bacc.Bacc("TRN2", target_bir_lowering=False, debug=False)

    xrows = nc.dram_tensor("xrows", [D, FAN, GROUP_PAD], _bf16,
                           kind="ExternalInput")
    nrows = nc.dram_tensor("nrows", [128, NTOTC], _bf16,
                           kind="ExternalInput")
    wfull = nc.dram_tensor("wfull", [D, FAN, CONV], _bf16,
                           kind="ExternalInput")
    waa = nc.dram_tensor("waa", [D, 128, CONV], _bf16, kind="ExternalInput")
    bsrow = nc.dram_tensor("bsrow", [D, CONV, 1], _f32, kind="ExternalInput")
    osort = nc.dram_tensor("osort", [CONV, NSORT], _bf16,
                           kind="ExternalOutput")

    with tile.TileContext(nc) as tc, ExitStack() as ctx:
        const_pool = ctx.enter_context(tc.tile_pool(name="const", bufs=1))
        x_pool = ctx.enter_context(tc.tile_pool(name="x", bufs=3))
        n_pool = ctx.enter_context(tc.tile_pool(name="n", bufs=2))
        o_pool = ctx.enter_context(tc.tile_pool(name="o", bufs=3))
        ps_pool = ctx.enter_context(tc.tile_pool(name="ps", bufs=8,
                                                 space="PSUM"))

        wt, wa2, bt = [], [], []
        for d in range(D):
            w = const_pool.tile([FAN, CONV], _bf16, tag=f"w{d}")
            nc.sync.dma_start(out=w[:], in_=wfull[d])
            wt.append(w)
            w2 = const_pool.tile([128, CONV], _bf16, tag=f"waa{d}")
            nc.sync.dma_start(out=w2[:], in_=waa[d])
            wa2.append(w2)
            b = const_pool.tile([CONV, 1], _f32, tag=f"b{d}")
            nc.sync.dma_start(out=b[:], in_=bsrow[d])
            bt.append(b)

        for g in range(D):
            d = D - 1 - g                  # group degree (DESC order)
            xg = x_pool.tile([FAN, GROUP_PAD], _bf16, tag="xg")
            nc.sync.dma_start(out=xg[:], in_=xrows[g])

            ng = None
            if d > 0:
                ng = n_pool.tile([128, NWIDTH[g]], _bf16, tag="ng")
                nc.sync.dma_start(
                    out=ng[:], in_=nrows[:, NOFF[g]:NOFF[g] + NWIDTH[g]])

            og = o_pool.tile([CONV, GROUP_PAD], _bf16, tag="og")
            for j in range(NCHUNK):
                c0, c1 = j * CHUNK, (j + 1) * CHUNK
                # (lhsT, rhs) accumulating into this chunk's bank
                mms = [(wt[d][:], xg[:, c0:c1])]
                for p in range(NPAIR[g] if "p" in _VARIANT else 0):
                    mms.append(
                        (wa2[d][:], ng[:, p * GROUP_PAD + c0:
                                       p * GROUP_PAD + c1]))

                ps = ps_pool.tile([CONV, CHUNK], _f32, tag="ps")
                for i, (lhsT, rhs) in enumerate(mms):
                    nc.tensor.matmul(
                        out=ps[:], lhsT=lhsT, rhs=rhs,
                        start=(i == 0), stop=(i == len(mms) - 1))
                nc.scalar.activation(
                    og[:, c0:c1], ps[:],
                    mybir.ActivationFunctionType.Relu, bias=bt[d][:])
            nc.scalar.dma_start(
                out=osort[:, g * GROUP_PAD:(g + 1) * GROUP_PAD], in_=og[:])

    nc.compile()
    return nc


def _get_program():
    if "nc" not in _cached:
        _cached["nc"] = build_program()
    return _cached["nc"]


def prep_core_inputs(atoms_s, bonds_s, edges_s, wfull_np, waa_np, bsrow_np):
    """Host-side layout/index prep for one core's shard (numpy only)."""
    deg = (edges_s != -1).sum(axis=-1).reshape(-1)            # [T] natural
    slot_tok = np.full(NSORT, -1, np.int64)   # sorted slot -> natural token
    for d in range(D):
        toks = np.nonzero(deg == d)[0]
        n = len(toks)
        assert n <= GROUP_PAD, f"degree-{d} group has {n} > {GROUP_PAD}"
        base = (D - 1 - d) * GROUP_PAD
        slot_tok[base:base + n] = toks

    flat_a = atoms_s.reshape(T, F_ATOM).astype(ml_dtypes.bfloat16)
    flat_b = bonds_s.reshape(T, D * F_BOND).astype(ml_dtypes.bfloat16)
    valid = slot_tok >= 0
    safe = np.maximum(slot_tok, 0)
    x = np.concatenate(
        [np.where(valid[:, None], flat_a[safe], ml_dtypes.bfloat16(0)),
         np.where(valid[:, None], flat_b[safe], ml_dtypes.bfloat16(0))],
        axis=1)                                               # [NSORT, 112]
    xrows = x.reshape(D, GROUP_PAD, FAN).transpose(0, 2, 1)   # [D,112,2560]

    eflat = edges_s.reshape(T, D)
    bcol = (np.arange(T) // A) * A                            # molecule base
    blocks = []
    for g in range(D - 1):
        d = D - 1 - g
        slots = slot_tok[g * GROUP_PAD:(g + 1) * GROUP_PAD]
        sv = np.maximum(slots, 0)
        slabs = []
        for s in range(d):
            e = np.where(slots >= 0, eflat[sv, s], -1)
            nat = np.maximum(bcol[sv] + e, 0)
            rows = np.where((e >= 0)[:, None], flat_a[nat],
                            ml_dtypes.bfloat16(0))            # [2560, 64]
            slabs.append(np.ascontiguousarray(rows.T))        # [64, 2560]
        blk = np.zeros((128, NWIDTH[g]), ml_dtypes.bfloat16)
        for p in range(NPAIR[g]):
            blk[0:64, p * GROUP_PAD:(p + 1) * GROUP_PAD] = slabs[2 * p]
            if 2 * p + 1 < d:
                blk[64:128, p * GROUP_PAD:(p + 1) * GROUP_PAD] = \
                    slabs[2 * p + 1]
        blocks.append(blk)
    nrows = np.concatenate(blocks, axis=1)                    # [128, 23040]

    return {
        "xrows": np.ascontiguousarray(xrows),
        "nrows": np.ascontiguousarray(nrows),
        "wfull": wfull_np,
        "waa": waa_np,
        "bsrow": bsrow_np,
    }, slot_tok


def kernel(atoms, bonds, edges, Ws, bs, trace=False):
    atoms = np.asarray(atoms)
    bonds = np.asarray(bonds)
    edges = np.asarray(edges)
    Ws = np.asarray(Ws)
    bs = np.asarray(bs)

    # Wfull rows = [Wa (64) | tile(Wb, 6) (48)]; bias via ACT bias AP
    wfull_np = np.zeros((D, FAN, CONV), np.float32)
    wfull_np[:, :F_ATOM] = Ws[:, :F_ATOM]
    wfull_np[:, F_ATOM:] = np.tile(Ws[:, F_ATOM:], (1, D, 1))
    wfull_np = wfull_np.astype(ml_dtypes.bfloat16)
    # duplicated atom weights for the partition-fold matmuls
    waa_np = np.concatenate([Ws[:, :F_ATOM], Ws[:, :F_ATOM]],
                            axis=1).astype(ml_dtypes.bfloat16)
    bsrow_np = np.ascontiguousarray(
        bs.reshape(D, CONV, 1).astype(np.float32))

    in_maps, slot_toks = [], []
    for c in range(NCORES):
        sl = slice(c * BS, (c + 1) * BS)
        m, st = prep_core_inputs(atoms[sl], bonds[sl], edges[sl],
                                 wfull_np, waa_np, bsrow_np)
        in_maps.append(m)
        slot_toks.append(st)

    nc = _get_program()
    res = run_bass_kernel_spmd(nc, in_maps, core_ids=list(range(NCORES)),
                               trace=trace)
    kernel.last_results = res

    out = np.zeros((B, A, CONV), np.float32)
    for c in range(NCORES):
        osort = res.results[c]["osort"].view(ml_dtypes.bfloat16)
        osort = osort.reshape(CONV, NSORT)                    # conv-major
        st = slot_toks[c]
        real = st >= 0
        shard = out[c * BS:(c + 1) * BS].reshape(T, CONV)
        shard[st[real]] = osort[:, real].T.astype(np.float32)
    return out


# revision 14
# speedup vs baseline: 2.9393x; 1.1129x over previous
"""Trainium2 Bass kernel for NeuralGraphHidden (GNN message passing).

Math (per molecule b, atom a):
    deg[b,a]    = #valid edges (edges[b,a,:] != -1)
    summed_atom = atoms[b,a] + sum_s atoms[b, edges[b,a,s]]          (64)
    bond_sum    = sum_s bonds[b,a,s]                                  (8)
    x           = concat(summed_atom, bond_sum)                      (72)
    out[b,a]    = relu(x @ Ws[deg] + bs[deg])  if deg <= 5 else 0   (128)

Design (v3 — feature-major layout, everything folds into PE accumulation):
  * Host does all *layout* work (degree-sort permutation, neighbour row
    expansion via np.take, transposition to [feature, token] order, bf16
    packing) — pure indexed data movement.  Device does all arithmetic.
  * Feature-major: tokens are matmul rhs columns, so NO on-device
    transposes.  out^T[conv, tok] = W_d^T @ x^T with W_d the stationary
    operand, N=512 moving tiles.
  * Bond features are shipped raw (48 = 6 slots x 8); the bond-slot sum
    folds into the matmul via W rows 64:112 = tile(Wb, 6).
  * Neighbour atom rows (degree-DESC sorted, slot-s list is a prefix)
    are packed two 64-row slabs per 128 partitions.  The slab summation
    folds into the SAME PSUM accumulation via duplicated atom weights:
      [top; bottom] @ [Wa_d; Wa_d] == (top + bottom) @ Wa_d
    (DVE can't add across partition halves — lanes are partition-locked
    — but the PE contraction dimension can.)  Odd slab counts zero-pad
    the last pair's bottom half (~1MB extra traffic) so every matmul is
    a uniform K=128 base-partition-0 full-bank accumulate — half-K
    base-64 row-group matmuls crash on HW in this stack.
  * Bias + relu + f32->bf16 fold into one ScalarE activation per chunk
    (bias is a per-partition [128,1] AP in the conv-major layout).
  * Per-degree groups padded to 2560 slots so all 8 cores share one
    SPMD program; host unpermutes the sorted output (deg-6 rows zero).
"""

import sys

sys.path.insert(0, "/opt/trn_rl_repo")

import numpy as np
import ml_dtypes

from contextlib import ExitStack

import concourse.bacc as bacc
import concourse.tile as tile
from concourse import mybir
from concourse.bass_utils import run_bass_kernel_spmd

# Problem shapes (hardcoded per the harness contract).
B, A, D = 1024, 128, 6
F_ATOM, F_BOND, CONV = 64, 8, 128
FAN = F_ATOM + D * F_BOND               # 112 features per packed column
NCORES = 8
BS = B // NCORES          # molecules per core = 128
T = BS * A                # tokens per core = 16384
GROUP_PAD = 2560                        # per-degree group size (static)
HALF = GROUP_PAD // 2                   # 1280
NSORT = D * GROUP_PAD                   # 15360 sorted slots
CHUNK = 512                             # matmul moving-tile width
NCHUNK = GROUP_PAD // CHUNK             # 5 chunks per degree group
# neighbour block for group g (degree d=5-g): ceil(d/2) pair sections
NPAIR = [(max(D - 1 - g, 0) + 1) // 2 for g in range(D)]
NWIDTH = [p * GROUP_PAD for p in NPAIR]
NOFF = [sum(NWIDTH[:g]) for g in range(D)]
NTOTC = sum(NWIDTH)                     # 23040 packed neighbour columns

_f32 = mybir.dt.float32
_bf16 = mybir.dt.bfloat16

_cached = {}

import os
_VARIANT = os.environ.get("K_VARIANT", "spt")  # s(elf) p(airs) t(ails)


def build_program():
    """Build the (static) per-core Bass/Tile program."""
    nc = bacc.Bacc("TRN2", target_bir_lowering=False, debug=False)

    xrows = nc.dram_tensor("xrows", [D, FAN, GROUP_PAD], _bf16,
                           kind="ExternalInput")
    nrows = nc.dram_tensor("nrows", [128, NTOTC], _bf16,
                           kind="ExternalInput")
    # all weights in one blob: per degree d, cols [d*256, d*256+128) =
    # wfull_d (rows 0:112), cols [d*256+128, d*256+256) = [Wa_d; Wa_d]
    wblob = nc.dram_tensor("wblob", [128, D * 2 * CONV], _bf16,
                           kind="ExternalInput")
    bsrow = nc.dram_tensor("bsrow", [CONV, D], _f32, kind="ExternalInput")
    osort = nc.dram_tensor("osort", [CONV, NSORT], _bf16,
                           kind="ExternalOutput")

    with tile.TileContext(nc) as tc, ExitStack() as ctx:
        const_pool = ctx.enter_context(tc.tile_pool(name="const", bufs=1))
        x_pool = ctx.enter_context(tc.tile_pool(name="x", bufs=4))
        n_pool = ctx.enter_context(tc.tile_pool(name="n", bufs=3))
        o_pool = ctx.enter_context(tc.tile_pool(name="o", bufs=4))
        ps_pool = ctx.enter_context(tc.tile_pool(name="ps", bufs=8,
                                                 space="PSUM"))

        wb = const_pool.tile([128, D * 2 * CONV], _bf16, tag="wb")
        nc.sync.dma_start(out=wb[:], in_=wblob[:])
        bias_t = const_pool.tile([CONV, D], _f32, tag="bias")
        nc.sync.dma_start(out=bias_t[:], in_=bsrow[:])

        for d in range(D):                 # ascending degree
            g = D - 1 - d                  # block index (DESC-sorted layout)
            xg = x_pool.tile([FAN, GROUP_PAD], _bf16, tag="xg")
            nc.sync.dma_start(out=xg[:], in_=xrows[g])

            ng = None
            if d > 0:
                ng = n_pool.tile([128, NWIDTH[g]], _bf16, tag="ng")
                nc.gpsimd.dma_start(
                    out=ng[:], in_=nrows[:, NOFF[g]:NOFF[g] + NWIDTH[g]])

            wt = wb[0:FAN, d * 2 * CONV:d * 2 * CONV + CONV]
            wa2 = wb[:, d * 2 * CONV + CONV:(d + 1) * 2 * CONV]
            og = o_pool.tile([CONV, GROUP_PAD], _bf16, tag="og")
            for j in range(NCHUNK):
                c0, c1 = j * CHUNK, (j + 1) * CHUNK
                # (lhsT, rhs) accumulating into this chunk's bank
                mms = [(wt, xg[:, c0:c1])]
                for p in range(NPAIR[g] if "p" in _VARIANT else 0):
                    mms.append(
                        (wa2, ng[:, p * GROUP_PAD + c0:
                                 p * GROUP_PAD + c1]))

                ps = ps_pool.tile([CONV, CHUNK], _f32, tag="ps")
                for i, (lhsT, rhs) in enumerate(mms):
                    nc.tensor.matmul(
                        out=ps[:], lhsT=lhsT, rhs=rhs,
                        start=(i == 0), stop=(i == len(mms) - 1))
                nc.scalar.activation(
                    og[:, c0:c1], ps[:],
                    mybir.ActivationFunctionType.Relu,
                    bias=bias_t[:, d:d + 1])
            nc.scalar.dma_start(
                out=osort[:, g * GROUP_PAD:(g + 1) * GROUP_PAD], in_=og[:])

    nc.compile()
    return nc


def _get_program():
    if "nc" not in _cached:
        _cached["nc"] = build_program()
    return _cached["nc"]


def prep_core_inputs(atoms_s, bonds_s, edges_s, wblob_np, bsrow_np):
    """Host-side layout/index prep for one core's shard (numpy only)."""
    deg = (edges_s != -1).sum(axis=-1).reshape(-1)            # [T] natural
    slot_tok = np.full(NSORT, -1, np.int64)   # sorted slot -> natural token
    for d in range(D):
        toks = np.nonzero(deg == d)[0]
        n = len(toks)
        assert n <= GROUP_PAD, f"degree-{d} group has {n} > {GROUP_PAD}"
        base = (D - 1 - d) * GROUP_PAD
        slot_tok[base:base + n] = toks

    flat_a = atoms_s.reshape(T, F_ATOM).astype(ml_dtypes.bfloat16)
    flat_b = bonds_s.reshape(T, D * F_BOND).astype(ml_dtypes.bfloat16)
    valid = slot_tok >= 0
    safe = np.maximum(slot_tok, 0)
    x = np.concatenate(
        [np.where(valid[:, None], flat_a[safe], ml_dtypes.bfloat16(0)),
         np.where(valid[:, None], flat_b[safe], ml_dtypes.bfloat16(0))],
        axis=1)                                               # [NSORT, 112]
    xrows = x.reshape(D, GROUP_PAD, FAN).transpose(0, 2, 1)   # [D,112,2560]

    eflat = edges_s.reshape(T, D)
    bcol = (np.arange(T) // A) * A                            # molecule base
    blocks = []
    for g in range(D - 1):
        d = D - 1 - g
        slots = slot_tok[g * GROUP_PAD:(g + 1) * GROUP_PAD]
        sv = np.maximum(slots, 0)
        slabs = []
        for s in range(d):
            e = np.where(slots >= 0, eflat[sv, s], -1)
            nat = np.maximum(bcol[sv] + e, 0)
            rows = np.where((e >= 0)[:, None], flat_a[nat],
                            ml_dtypes.bfloat16(0))            # [2560, 64]
            slabs.append(np.ascontiguousarray(rows.T))        # [64, 2560]
        blk = np.zeros((128, NWIDTH[g]), ml_dtypes.bfloat16)
        for p in range(NPAIR[g]):
            blk[0:64, p * GROUP_PAD:(p + 1) * GROUP_PAD] = slabs[2 * p]
            if 2 * p + 1 < d:
                blk[64:128, p * GROUP_PAD:(p + 1) * GROUP_PAD] = \
                    slabs[2 * p + 1]
        blocks.append(blk)
    nrows = np.concatenate(blocks, axis=1)                    # [128, 23040]

    return {
        "xrows": np.ascontiguousarray(xrows),
        "nrows": np.ascontiguousarray(nrows),
        "wblob": wblob_np,
        "bsrow": bsrow_np,
    }, slot_tok


def kernel(atoms, bonds, edges, Ws, bs, trace=False):
    atoms = np.asarray(atoms)
    bonds = np.asarray(bonds)
    edges = np.asarray(edges)
    Ws = np.asarray(Ws)
    bs = np.asarray(bs)

    # Wfull rows = [Wa (64) | tile(Wb, 6) (48)]; bias via ACT bias AP
    wblob_np = np.zeros((128, D * 2 * CONV), np.float32)
    for d in range(D):
        wblob_np[:F_ATOM, d * 2 * CONV:d * 2 * CONV + CONV] = Ws[d, :F_ATOM]
        wblob_np[F_ATOM:FAN, d * 2 * CONV:d * 2 * CONV + CONV] = \
            np.tile(Ws[d, F_ATOM:], (D, 1))
        # duplicated atom weights for the partition-fold matmuls
        wblob_np[0:64, d * 2 * CONV + CONV:(d + 1) * 2 * CONV] = \
            Ws[d, :F_ATOM]
        wblob_np[64:128, d * 2 * CONV + CONV:(d + 1) * 2 * CONV] = \
            Ws[d, :F_ATOM]
    wblob_np = wblob_np.astype(ml_dtypes.bfloat16)
    bsrow_np = np.ascontiguousarray(bs.T.astype(np.float32))  # [CONV, D]

    in_maps, slot_toks = [], []
    for c in range(NCORES):
        sl = slice(c * BS, (c + 1) * BS)
        m, st = prep_core_inputs(atoms[sl], bonds[sl], edges[sl],
                                 wblob_np, bsrow_np)
        in_maps.append(m)
        slot_toks.append(st)

    nc = _get_program()
    res = run_bass_kernel_spmd(nc, in_maps, core_ids=list(range(NCORES)),
                               trace=trace)
    kernel.last_results = res

    out = np.zeros((B, A, CONV), np.float32)
    for c in range(NCORES):
        osort = res.results[c]["osort"].view(ml_dtypes.bfloat16)
        osort = osort.reshape(CONV, NSORT)                    # conv-major
        st = slot_toks[c]
        real = st >= 0
        shard = out[c * BS:(c + 1) * BS].reshape(T, CONV)
        shard[st[real]] = osort[:, real].T.astype(np.float32)
    return out


# revision 17
# speedup vs baseline: 3.2341x; 1.1003x over previous
"""Trainium2 Bass kernel for NeuralGraphHidden (GNN message passing).

Math (per molecule b, atom a):
    deg[b,a]    = #valid edges (edges[b,a,:] != -1)
    summed_atom = atoms[b,a] + sum_s atoms[b, edges[b,a,s]]          (64)
    bond_sum    = sum_s bonds[b,a,s]                                  (8)
    x           = concat(summed_atom, bond_sum)                      (72)
    out[b,a]    = relu(x @ Ws[deg] + bs[deg])  if deg <= 5 else 0   (128)

Design (v3 — feature-major layout, everything folds into PE accumulation):
  * Host does all *layout* work (degree-sort permutation, neighbour row
    expansion via np.take, transposition to [feature, token] order, bf16
    packing) — pure indexed data movement.  Device does all arithmetic.
  * Feature-major: tokens are matmul rhs columns, so NO on-device
    transposes.  out^T[conv, tok] = W_d^T @ x^T with W_d the stationary
    operand, N=512 moving tiles.
  * Bond features are shipped raw (48 = 6 slots x 8); the bond-slot sum
    folds into the matmul via W rows 64:112 = tile(Wb, 6).
  * Neighbour atom rows (degree-DESC sorted, slot-s list is a prefix)
    are packed two 64-row slabs per 128 partitions.  The slab summation
    folds into the SAME PSUM accumulation via duplicated atom weights:
      [top; bottom] @ [Wa_d; Wa_d] == (top + bottom) @ Wa_d
    (DVE can't add across partition halves — lanes are partition-locked
    — but the PE contraction dimension can.)  Odd slab counts zero-pad
    the last pair's bottom half (~1MB extra traffic) so every matmul is
    a uniform K=128 base-partition-0 full-bank accumulate — half-K
    base-64 row-group matmuls crash on HW in this stack.
  * Bias + relu + f32->bf16 fold into one ScalarE activation per chunk
    (bias is a per-partition [128,1] AP in the conv-major layout).
  * Per-degree groups padded to 2560 slots so all 8 cores share one
    SPMD program; host unpermutes the sorted output (deg-6 rows zero).
"""

import sys

sys.path.insert(0, "/opt/trn_rl_repo")

import numpy as np
import ml_dtypes

from contextlib import ExitStack

import concourse.bacc as bacc
import concourse.tile as tile
from concourse import mybir
from concourse.bass_utils import run_bass_kernel_spmd

# Problem shapes (hardcoded per the harness contract).
B, A, D = 1024, 128, 6
F_ATOM, F_BOND, CONV = 64, 8, 128
FAN = F_ATOM + D * F_BOND               # 112 features per packed column
NCORES = 8
BS = B // NCORES          # molecules per core = 128
T = BS * A                # tokens per core = 16384
GROUP_PAD = 2560                        # per-degree group size (static)
HALF = GROUP_PAD // 2                   # 1280
NSORT = D * GROUP_PAD                   # 15360 sorted slots
CHUNK = 512                             # matmul moving-tile width
NCHUNK = GROUP_PAD // CHUNK             # 5 chunks per degree group
# neighbour block for group g (degree d=5-g): ceil(d/2) pair sections
NPAIR = [(max(D - 1 - g, 0) + 1) // 2 for g in range(D)]
NWIDTH = [p * GROUP_PAD for p in NPAIR]
NOFF = [sum(NWIDTH[:g]) for g in range(D)]
NTOTC = sum(NWIDTH)                     # 23040 packed neighbour columns

_f32 = mybir.dt.float32
_bf16 = mybir.dt.bfloat16

_cached = {}

import os
_VARIANT = os.environ.get("K_VARIANT", "spt")  # s(elf) p(airs) t(ails)


def build_program():
    """Build the (static) per-core Bass/Tile program."""
    nc = bacc.Bacc("TRN2", target_bir_lowering=False, debug=False)

    xrows = nc.dram_tensor("xrows", [D, FAN, GROUP_PAD], _bf16,
                           kind="ExternalInput")
    nrows = nc.dram_tensor("nrows", [128, NTOTC], _bf16,
                           kind="ExternalInput")
    # all weights in one blob: per degree d, cols [d*256, d*256+128) =
    # wfull_d (rows 0:112), cols [d*256+128, d*256+256) = [Wa_d; Wa_d]
    wblob = nc.dram_tensor("wblob", [128, D * 2 * CONV], _bf16,
                           kind="ExternalInput")
    bsrow = nc.dram_tensor("bsrow", [CONV, D], _f32, kind="ExternalInput")
    osort = nc.dram_tensor("osort", [CONV, NSORT], _bf16,
                           kind="ExternalOutput")

    with tile.TileContext(nc) as tc, ExitStack() as ctx:
        const_pool = ctx.enter_context(tc.tile_pool(name="const", bufs=1))
        x_pool = ctx.enter_context(tc.tile_pool(name="x", bufs=1))
        n_pool = ctx.enter_context(tc.tile_pool(name="n", bufs=1))
        o_pool = ctx.enter_context(tc.tile_pool(name="o", bufs=1))
        ps_pool = ctx.enter_context(tc.tile_pool(name="ps", bufs=8,
                                                 space="PSUM"))

        wb = const_pool.tile([128, D * 2 * CONV], _bf16, tag="wb")
        nc.sync.dma_start(out=wb[:], in_=wblob[:])
        bias_t = const_pool.tile([CONV, D], _f32, tag="bias")
        nc.sync.dma_start(out=bias_t[:], in_=bsrow[:])

        for d in range(D):                 # ascending degree
            g = D - 1 - d                  # block index (DESC-sorted layout)
            xg = x_pool.tile([FAN, GROUP_PAD], _bf16, tag=f"xg{d}")
            nc.sync.dma_start(out=xg[:], in_=xrows[g])

            ng = None
            if d > 0:
                ng = n_pool.tile([128, NWIDTH[g]], _bf16, tag=f"ng{d}")
                nc.sync.dma_start(
                    out=ng[:], in_=nrows[:, NOFF[g]:NOFF[g] + NWIDTH[g]])

            wt = wb[0:FAN, d * 2 * CONV:d * 2 * CONV + CONV]
            wa2 = wb[:, d * 2 * CONV + CONV:(d + 1) * 2 * CONV]
            og = o_pool.tile([CONV, GROUP_PAD], _bf16, tag=f"og{d}")
            for j in range(NCHUNK):
                c0, c1 = j * CHUNK, (j + 1) * CHUNK
                # (lhsT, rhs) accumulating into this chunk's bank
                mms = [(wt, xg[:, c0:c1])]
                for p in range(NPAIR[g] if "p" in _VARIANT else 0):
                    mms.append(
                        (wa2, ng[:, p * GROUP_PAD + c0:
                                 p * GROUP_PAD + c1]))

                ps = ps_pool.tile([CONV, CHUNK], _f32, tag="ps")
                for i, (lhsT, rhs) in enumerate(mms):
                    nc.tensor.matmul(
                        out=ps[:], lhsT=lhsT, rhs=rhs,
                        start=(i == 0), stop=(i == len(mms) - 1))
                nc.scalar.activation(
                    og[:, c0:c1], ps[:],
                    mybir.ActivationFunctionType.Relu,
                    bias=bias_t[:, d:d + 1])
            nc.scalar.dma_start(
                out=osort[:, g * GROUP_PAD:(g + 1) * GROUP_PAD], in_=og[:])

    nc.compile()
    return nc


def _get_program():
    if "nc" not in _cached:
        _cached["nc"] = build_program()
    return _cached["nc"]


def prep_core_inputs(atoms_s, bonds_s, edges_s, wblob_np, bsrow_np):
    """Host-side layout/index prep for one core's shard (numpy only)."""
    deg = (edges_s != -1).sum(axis=-1).reshape(-1)            # [T] natural
    slot_tok = np.full(NSORT, -1, np.int64)   # sorted slot -> natural token
    for d in range(D):
        toks = np.nonzero(deg == d)[0]
        n = len(toks)
        assert n <= GROUP_PAD, f"degree-{d} group has {n} > {GROUP_PAD}"
        base = (D - 1 - d) * GROUP_PAD
        slot_tok[base:base + n] = toks

    flat_a = atoms_s.reshape(T, F_ATOM).astype(ml_dtypes.bfloat16)
    flat_b = bonds_s.reshape(T, D * F_BOND).astype(ml_dtypes.bfloat16)
    valid = slot_tok >= 0
    safe = np.maximum(slot_tok, 0)
    x = np.concatenate(
        [np.where(valid[:, None], flat_a[safe], ml_dtypes.bfloat16(0)),
         np.where(valid[:, None], flat_b[safe], ml_dtypes.bfloat16(0))],
        axis=1)                                               # [NSORT, 112]
    xrows = x.reshape(D, GROUP_PAD, FAN).transpose(0, 2, 1)   # [D,112,2560]

    eflat = edges_s.reshape(T, D)
    bcol = (np.arange(T) // A) * A                            # molecule base
    blocks = []
    for g in range(D - 1):
        d = D - 1 - g
        slots = slot_tok[g * GROUP_PAD:(g + 1) * GROUP_PAD]
        sv = np.maximum(slots, 0)
        slabs = []
        for s in range(d):
            e = np.where(slots >= 0, eflat[sv, s], -1)
            nat = np.maximum(bcol[sv] + e, 0)
            rows = np.where((e >= 0)[:, None], flat_a[nat],
                            ml_dtypes.bfloat16(0))            # [2560, 64]
            slabs.append(np.ascontiguousarray(rows.T))        # [64, 2560]
        blk = np.zeros((128, NWIDTH[g]), ml_dtypes.bfloat16)
        for p in range(NPAIR[g]):
            blk[0:64, p * GROUP_PAD:(p + 1) * GROUP_PAD] = slabs[2 * p]
            if 2 * p + 1 < d:
                blk[64:128, p * GROUP_PAD:(p + 1) * GROUP_PAD] = \
                    slabs[2 * p + 1]
        blocks.append(blk)
    nrows = np.concatenate(blocks, axis=1)                    # [128, 23040]

    return {
        "xrows": np.ascontiguousarray(xrows),
        "nrows": np.ascontiguousarray(nrows),
        "wblob": wblob_np,
        "bsrow": bsrow_np,
    }, slot_tok


def kernel(atoms, bonds, edges, Ws, bs, trace=False):
    atoms = np.asarray(atoms)
    bonds = np.asarray(bonds)
    edges = np.asarray(edges)
    Ws = np.asarray(Ws)
    bs = np.asarray(bs)

    # Wfull rows = [Wa (64) | tile(Wb, 6) (48)]; bias via ACT bias AP
    wblob_np = np.zeros((128, D * 2 * CONV), np.float32)
    for d in range(D):
        wblob_np[:F_ATOM, d * 2 * CONV:d * 2 * CONV + CONV] = Ws[d, :F_ATOM]
        wblob_np[F_ATOM:FAN, d * 2 * CONV:d * 2 * CONV + CONV] = \
            np.tile(Ws[d, F_ATOM:], (D, 1))
        # duplicated atom weights for the partition-fold matmuls
        wblob_np[0:64, d * 2 * CONV + CONV:(d + 1) * 2 * CONV] = \
            Ws[d, :F_ATOM]
        wblob_np[64:128, d * 2 * CONV + CONV:(d + 1) * 2 * CONV] = \
            Ws[d, :F_ATOM]
    wblob_np = wblob_np.astype(ml_dtypes.bfloat16)
    bsrow_np = np.ascontiguousarray(bs.T.astype(np.float32))  # [CONV, D]

    in_maps, slot_toks = [], []
    for c in range(NCORES):
        sl = slice(c * BS, (c + 1) * BS)
        m, st = prep_core_inputs(atoms[sl], bonds[sl], edges[sl],
                                 wblob_np, bsrow_np)
        in_maps.append(m)
        slot_toks.append(st)

    nc = _get_program()
    res = run_bass_kernel_spmd(nc, in_maps, core_ids=list(range(NCORES)),
                               trace=trace)
    kernel.last_results = res

    out = np.zeros((B, A, CONV), np.float32)
    for c in range(NCORES):
        osort = res.results[c]["osort"].view(ml_dtypes.bfloat16)
        osort = osort.reshape(CONV, NSORT)                    # conv-major
        st = slot_toks[c]
        real = st >= 0
        shard = out[c * BS:(c + 1) * BS].reshape(T, CONV)
        shard[st[real]] = osort[:, real].T.astype(np.float32)
    return out
